# revision 1
# baseline (speedup 1.0000x reference)
"""TRN2 Bass kernel for nn_Mix2Layer (dense MLP mixture).

Reference computation (all fp32):
    g   = relu(einsum('bi,iok->bok', x, w1) + b1)        # [B, DOUT, K]
    out = einsum('bi,iok,bok->bo', x, w2, g) + b2        # [B, DOUT]

Strategy: tensor-parallel over DOUT across 8 NeuronCores (each core owns a
256-wide dout shard; the bok intermediate never leaves its core). On each
core both einsums are plain matmuls of x [B, DIN] against the shard's
weights flattened to [DIN, DS*K], run on the PE array in float32r — the
PE fast path for 4-byte floats (1 cycle/row when the moving dim is >=256,
i.e. bf16-speed). float32r keeps 11 explicit mantissa bits (measured on
hardware: round-to-nearest-even at 11 bits on both operands reproduces
the PE result to 1e-7), giving ~1e-4 relative error overall.

The matmul operands are pre-rounded to the fp32r grid on the HOST, so all
tensors are declared float32r end-to-end and every DMA is a fast same-dtype
HWDGE transfer (the gpsimd cast-DMA path measured ~100x below line rate).

Per-core loop structure (B split in halves to bound SBUF):
  for b_half:                      # xT half resident in SBUF (64 KB/part)
    for ok_chunk (256 cols):       # w1/w2 chunk tiles double-buffered
      for b_tile (8 x 128 rows):
        psum_h1 = sum_i xT_i.T @ w1_chunk_i    (16 fp32r matmuls)
        psum_h2 = sum_i xT_i.T @ w2_chunk_i
        h2s     = copy(psum_h2)                 (ScalarE)
        p       = relu(psum_h1) * h2s           (VectorE, one fused op)
        acc[b_tile][:, chunk] = reduce_k(p)     (VectorE, 3D-AP reduce)
    DMA acc tiles -> out rows
"""
import numpy as np

import concourse.bass as bass
import concourse.tile as tile
import concourse.mybir as mybir
from concourse import bacc
from concourse.bass_interp import get_hw_module
from concourse.bass_utils import run_bass_kernel_spmd

P = 128
f32 = mybir.dt.float32
f32r = mybir.dt.float32r

N_CORES = 8
F32R_MANT_BITS = 11


def round_f32r(a):
    """Round fp32 array to the fp32r grid (11 explicit mantissa bits, RNE)."""
    a = np.ascontiguousarray(a, dtype=np.float32)
    bits = a.view(np.uint32)
    shift = np.uint32(23 - F32R_MANT_BITS)
    lsb = np.uint32(1) << shift
    half = lsb >> np.uint32(1)
    rounded = (bits + half + ((bits >> shift) & np.uint32(1)) - np.uint32(1)) \
        & ~np.uint32(lsb - np.uint32(1))
    return rounded.view(np.float32)


def build_program(din, b, dout_s, k, with_b1, with_b2, ch=256, n_half=2,
                  num_devices=N_CORES):
    """Build + schedule + compile the per-core Bass program.

    din: contraction dim; b: batch; dout_s: per-core dout shard; k: mixture.
    ch: ok-chunk width (matmul free dim; >=256 keeps fp32r at 1 cyc/row).
    """
    okw = dout_s * k
    assert din % P == 0 and b % (P * n_half) == 0 and okw % ch == 0
    assert ch % k == 0
    it_n = din // P
    bh = b // n_half
    nbt = bh // P
    nch = okw // ch
    o_ch = ch // k

    nc = bacc.Bacc("TRN2", target_bir_lowering=False, debug=False,
                   enable_asserts=True, num_devices=num_devices)
    xt_d = nc.dram_tensor("xt", [din, b], f32r, kind="ExternalInput").ap()
    w1_d = nc.dram_tensor("w1s", [din, okw], f32r, kind="ExternalInput").ap()
    w2_d = nc.dram_tensor("w2s", [din, okw], f32r, kind="ExternalInput").ap()
    b1_d = nc.dram_tensor("b1s", [okw], f32r, kind="ExternalInput").ap()
    b2_d = nc.dram_tensor("b2s", [dout_s], f32, kind="ExternalInput").ap()
    out_d = nc.dram_tensor("out", [b, dout_s], f32, kind="ExternalOutput").ap()

    # DRAM views with i-tiles split out: [p, it, cols]
    xt_v = xt_d.rearrange("(t p) b -> p t b", p=P)
    w1_v = w1_d.rearrange("(t p) n -> p t n", p=P)
    w2_v = w2_d.rearrange("(t p) n -> p t n", p=P)

    from contextlib import ExitStack
    with tile.TileContext(nc) as tc, ExitStack() as ctx:
        xt_pool = ctx.enter_context(tc.tile_pool(name="xt", bufs=1))
        w_pool = ctx.enter_context(tc.tile_pool(name="w", bufs=2))
        acc_pool = ctx.enter_context(tc.tile_pool(name="acc", bufs=2 * nbt))
        ep_pool = ctx.enter_context(tc.tile_pool(name="ep", bufs=3))
        const_pool = ctx.enter_context(tc.tile_pool(name="const", bufs=1))
        ps1_pool = ctx.enter_context(
            tc.tile_pool(name="ps1", bufs=3, space="PSUM"))
        ps2_pool = ctx.enter_context(
            tc.tile_pool(name="ps2", bufs=3, space="PSUM"))

        if with_b1:
            ones_t = const_pool.tile([1, P], f32r, tag="ones")
            nc.any.memset(ones_t[:], 1.0)
        if with_b2:
            b2bc = const_pool.tile([P, dout_s], f32, tag="b2bc")
            nc.gpsimd.dma_start(b2bc[:],
                                b2_d[None, :].broadcast_to([P, dout_s]))

        for bhid in range(n_half):
            xt_t = xt_pool.tile([P, it_n, bh], f32r, tag="xth")
            nc.sync.dma_start(
                xt_t[:], xt_v[:, :, bhid * bh:(bhid + 1) * bh])

            accs = [acc_pool.tile([P, dout_s], f32, tag="acc",
                                  name=f"acc_{bhid}_{i}")
                    for i in range(nbt)]

            for c in range(nch):
                w1_t = w_pool.tile([P, it_n, ch], f32r, tag="w1c")
                nc.sync.dma_start(
                    w1_t[:], w1_v[:, :, c * ch:(c + 1) * ch])
                w2_t = w_pool.tile([P, it_n, ch], f32r, tag="w2c")
                nc.sync.dma_start(
                    w2_t[:], w2_v[:, :, c * ch:(c + 1) * ch])
                if with_b1:
                    b1r = ep_pool.tile([1, ch], f32r, tag="b1r")
                    nc.sync.dma_start(
                        b1r[:], b1_d[None, c * ch:(c + 1) * ch])

                for bt in range(nbt):
                    xt_sl = xt_t[:, :, bt * P:(bt + 1) * P]
                    ph1 = ps1_pool.tile([P, ch], f32, tag="ph1")
                    for it in range(it_n):
                        nc.tensor.matmul(
                            ph1[:],
                            lhsT=xt_sl[:, it, :],
                            rhs=w1_t[:, it, :],
                            start=(it == 0),
                            stop=(it == it_n - 1 and not with_b1),
                        )
                    if with_b1:
                        nc.tensor.matmul(ph1[:], lhsT=ones_t[:], rhs=b1r[:],
                                         start=False, stop=True)
                    ph2 = ps2_pool.tile([P, ch], f32, tag="ph2")
                    for it in range(it_n):
                        nc.tensor.matmul(
                            ph2[:],
                            lhsT=xt_sl[:, it, :],
                            rhs=w2_t[:, it, :],
                            start=(it == 0),
                            stop=(it == it_n - 1),
                        )

                    h2s = ep_pool.tile([P, ch], f32, tag="h2s")
                    nc.scalar.copy(h2s[:], ph2[:])
                    p_t = ep_pool.tile([P, ch], f32, tag="pt")
                    # p = relu(h1) * h2  in one DVE op
                    nc.vector.scalar_tensor_tensor(
                        out=p_t[:], in0=ph1[:], scalar=0.0, in1=h2s[:],
                        op0=mybir.AluOpType.max, op1=mybir.AluOpType.mult)
                    nc.vector.tensor_reduce(
                        out=accs[bt][:, c * o_ch:(c + 1) * o_ch],
                        in_=p_t[:].rearrange("p (o k) -> p o k", k=k),
                        axis=mybir.AxisListType.X,
                        op=mybir.AluOpType.add)

            for bt in range(nbt):
                if with_b2:
                    nc.vector.tensor_add(accs[bt][:], accs[bt][:], b2bc[:])
                nc.sync.dma_start(
                    out_d[bhid * bh + bt * P: bhid * bh + (bt + 1) * P, :],
                    accs[bt][:])

    nc.compile()
    nc.m = get_hw_module(nc.m)
    return nc


def shard_inputs(x, w1, b1, w2, b2, n_cores=N_CORES):
    b_dim, din = x.shape
    _, dout, k = w1.shape
    ds = dout // n_cores
    xt = round_f32r(np.ascontiguousarray(np.asarray(x, np.float32).T))
    in_maps = []
    for c in range(n_cores):
        sl = slice(c * ds, (c + 1) * ds)
        in_maps.append({
            "xt": xt,
            "w1s": round_f32r(w1[:, sl, :]).reshape(din, ds * k),
            "w2s": round_f32r(w2[:, sl, :]).reshape(din, ds * k),
            "b1s": round_f32r(b1[sl, :]).reshape(ds * k),
            "b2s": np.ascontiguousarray(b2[sl], dtype=np.float32),
        })
    return in_maps


_PROGRAM_CACHE = {}


def _get_program(din, b, dout_s, k, with_b1, with_b2):
    key = (din, b, dout_s, k, with_b1, with_b2)
    if key not in _PROGRAM_CACHE:
        _PROGRAM_CACHE[key] = build_program(
            din, b, dout_s, k, with_b1, with_b2)
    return _PROGRAM_CACHE[key]


class ParallelRunner:
    """Dispatch the per-core NEFF to each NeuronCore via its own jit so the
    8 executions overlap. (run_bass_kernel_spmd's shard_map path serializes
    the per-device executes through the axon proxy — measured 8x slower
    wall-clock for identical device work.)"""

    def __init__(self, nc, n_cores=N_CORES):
        import jax
        from concourse import bass2jax
        bass2jax.install_neuronx_cc_hook()
        self.jax = jax
        self.n_cores = n_cores
        part = nc.partition_id_tensor.name if nc.partition_id_tensor else None

        in_names, out_names, out_avals, zero_outs = [], [], [], []
        for alloc in nc.m.functions[0].allocations:
            if not isinstance(alloc, mybir.MemoryLocationSet):
                continue
            name = alloc.memorylocations[0].name
            if alloc.kind == "ExternalInput":
                if name != part:
                    in_names.append(name)
            elif alloc.kind == "ExternalOutput":
                out_names.append(name)
                shape = tuple(alloc.tensor_shape)
                dtype = mybir.dt.np(alloc.dtype)
                out_avals.append(jax.core.ShapedArray(shape, dtype))
                zero_outs.append(np.zeros(shape, dtype))
        self.in_names, self.out_names = in_names, out_names
        all_names = in_names + out_names + ([part] if part else [])

        def _body(*args):
            operands = list(args)
            if part is not None:
                operands.append(bass2jax.partition_id_tensor())
            return tuple(bass2jax._bass_exec_p.bind(
                *operands,
                out_avals=tuple(out_avals),
                in_names=tuple(all_names),
                out_names=tuple(out_names),
                lowering_input_output_aliases=(),
                sim_require_finite=True,
                sim_require_nnan=True,
                nc=nc,
            ))

        self.devices = jax.devices()[:n_cores]
        self.fns = [jax.jit(_body, device=d, keep_unused=True)
                    for d in self.devices]
        self.zero_dev = [
            [jax.device_put(z, d) for z in zero_outs] for d in self.devices]

    def __call__(self, in_maps):
        outs = []
        for c in range(self.n_cores):
            args = [self.jax.device_put(np.asarray(in_maps[c][n]),
                                        self.devices[c])
                    for n in self.in_names]
            outs.append(self.fns[c](*args, *self.zero_dev[c]))
        self.jax.block_until_ready(outs)
        return [{n: np.asarray(outs[c][i])
                 for i, n in enumerate(self.out_names)}
                for c in range(self.n_cores)]


_RUNNER_CACHE = {}


def _run(nc, in_maps):
    key = id(nc)
    try:
        if key not in _RUNNER_CACHE:
            _RUNNER_CACHE[key] = ParallelRunner(nc)
        return _RUNNER_CACHE[key](in_maps)
    except Exception:
        res = run_bass_kernel_spmd(nc, in_maps,
                                   core_ids=list(range(N_CORES)))
        return res.results


def kernel(x, w1, b1, w2, b2):
    x = np.asarray(x, dtype=np.float32)
    w1 = np.asarray(w1, dtype=np.float32)
    b1 = np.asarray(b1, dtype=np.float32)
    w2 = np.asarray(w2, dtype=np.float32)
    b2 = np.asarray(b2, dtype=np.float32)

    b_dim, din = x.shape
    _, dout, k = w1.shape
    ds = dout // N_CORES

    nc = _get_program(din, b_dim, ds, k,
                      bool(np.any(b1)), bool(np.any(b2)))
    in_maps = shard_inputs(x, w1, b1, w2, b2)
    results = _run(nc, in_maps)
    out = np.concatenate(
        [results[c]["out"] for c in range(N_CORES)], axis=1)
    return np.ascontiguousarray(out, dtype=np.float32)



# revision 2
# speedup vs baseline: 79.5775x; 79.5775x over previous
"""TRN2 Bass kernel for nn_Mix2Layer (dense MLP mixture).

Reference computation (all fp32):
    g   = relu(einsum('bi,iok->bok', x, w1) + b1)        # [B, DOUT, K]
    out = einsum('bi,iok,bok->bo', x, w2, g) + b2        # [B, DOUT]

Strategy: 2x4 grid over the 8 NeuronCores — batch B split in 2 groups of
1024 rows, DOUT split in 4 shards of 512 (the bok intermediate never
leaves its core). On each core both einsums are plain matmuls of the
core's x rows [1024, DIN] against the shard's weights flattened to
[DIN, DS*K], run on the PE array in float32r — the PE fast path for
4-byte floats (1 cycle/row when the moving dim is >=256, i.e.
bf16-speed). float32r keeps 11 explicit mantissa bits (measured on
hardware: round-to-nearest-even at 11 bits on both operands reproduces
the PE result to 1e-7), giving ~2e-4 relative error overall.

The matmul operands are pre-rounded to the fp32r grid on the HOST, so all
tensors are declared float32r end-to-end and every DMA is a fast same-dtype
HWDGE transfer (the gpsimd cast-DMA path measured ~100x below line rate).

vs the earlier 1x8 dout-only sharding (B processed in two halves per
core): the xT shard (64 KB/partition) now stays resident for the whole
kernel, which removes the 31 us PE stall at the half boundary (xT
reload) seen in the NTFF trace, and the xT load is split per b-tile so
the first matmuls start ~10 us in instead of ~44 us.

Per-core loop structure:
  xt_bt[8] <- xT b-tiles (8 x 1 MB DMAs, overlap chunk-0 weight DMAs)
  for ok_chunk (256 cols of DS*K=8192):   # w1/w2 chunk tiles double-buffered
    for b_tile (8 x 128 rows):
      psum_h1 = sum_i xt_bt.T @ w1_chunk_i   (16 fp32r matmuls)
      psum_h2 = sum_i xt_bt.T @ w2_chunk_i
      h2s     = copy(psum_h2)                 (ScalarE)
      p       = relu(psum_h1) * h2s           (VectorE, one fused op)
      acc[b_tile][:, chunk] = reduce_k(p)     (VectorE, 3D-AP reduce)
  DMA acc tiles -> out rows
"""
import numpy as np

import concourse.bass as bass
import concourse.tile as tile
import concourse.mybir as mybir
from concourse import bacc
from concourse.bass_interp import get_hw_module
from concourse.bass_utils import run_bass_kernel_spmd

P = 128
f32 = mybir.dt.float32
f32r = mybir.dt.float32r

N_CORES = 8
B_GROUPS = 2   # batch split across cores
D_GROUPS = 4   # dout split across cores
F32R_MANT_BITS = 11


def round_f32r(a):
    """Round fp32 array to the fp32r grid (11 explicit mantissa bits, RNE)."""
    a = np.ascontiguousarray(a, dtype=np.float32)
    bits = a.view(np.uint32)
    shift = np.uint32(23 - F32R_MANT_BITS)
    lsb = np.uint32(1) << shift
    half = lsb >> np.uint32(1)
    rounded = (bits + half + ((bits >> shift) & np.uint32(1)) - np.uint32(1)) \
        & ~np.uint32(lsb - np.uint32(1))
    return rounded.view(np.float32)


def build_program(din, b, dout_s, k, with_b1, with_b2, ch=256,
                  num_devices=N_CORES):
    """Build + schedule + compile the per-core Bass program.

    din: contraction dim; b: per-core batch rows; dout_s: per-core dout
    shard; k: mixture. ch: ok-chunk width (matmul free dim; >=256 keeps
    fp32r at 1 cyc/row).
    """
    okw = dout_s * k
    assert din % P == 0 and b % P == 0 and okw % ch == 0
    assert ch % k == 0
    it_n = din // P
    nbt = b // P
    nch = okw // ch
    o_ch = ch // k

    nc = bacc.Bacc("TRN2", target_bir_lowering=False, debug=False,
                   enable_asserts=True, num_devices=num_devices)
    xt_d = nc.dram_tensor("xt", [din, b], f32r, kind="ExternalInput").ap()
    w1_d = nc.dram_tensor("w1s", [din, okw], f32r, kind="ExternalInput").ap()
    w2_d = nc.dram_tensor("w2s", [din, okw], f32r, kind="ExternalInput").ap()
    b1_d = nc.dram_tensor("b1s", [okw], f32r, kind="ExternalInput").ap()
    b2_d = nc.dram_tensor("b2s", [dout_s], f32, kind="ExternalInput").ap()
    out_d = nc.dram_tensor("out", [b, dout_s], f32, kind="ExternalOutput").ap()

    # DRAM views with i-tiles split out: [p, it, cols]
    xt_v = xt_d.rearrange("(t p) b -> p t b", p=P)
    w1_v = w1_d.rearrange("(t p) n -> p t n", p=P)
    w2_v = w2_d.rearrange("(t p) n -> p t n", p=P)

    from contextlib import ExitStack
    with tile.TileContext(nc) as tc, ExitStack() as ctx:
        xt_pool = ctx.enter_context(tc.tile_pool(name="xt", bufs=nbt))
        w_pool = ctx.enter_context(tc.tile_pool(name="w", bufs=2))
        acc_pool = ctx.enter_context(tc.tile_pool(name="acc", bufs=nbt))
        ep_pool = ctx.enter_context(tc.tile_pool(name="ep", bufs=3))
        const_pool = ctx.enter_context(tc.tile_pool(name="const", bufs=1))
        ps1_pool = ctx.enter_context(
            tc.tile_pool(name="ps1", bufs=3, space="PSUM"))
        ps2_pool = ctx.enter_context(
            tc.tile_pool(name="ps2", bufs=3, space="PSUM"))

        if with_b1:
            ones_t = const_pool.tile([1, P], f32r, tag="ones")
            nc.any.memset(ones_t[:], 1.0)
        if with_b2:
            b2bc = const_pool.tile([P, dout_s], f32, tag="b2bc")
            nc.gpsimd.dma_start(b2bc[:],
                                b2_d[None, :].broadcast_to([P, dout_s]))

        # xT resident for the whole kernel, loaded per b-tile so chunk-0
        # matmuls start as soon as their own slice lands.
        xts = []
        for bt in range(nbt):
            xt_t = xt_pool.tile([P, it_n, P], f32r, tag="xtb",
                                name=f"xt_{bt}")
            nc.sync.dma_start(xt_t[:], xt_v[:, :, bt * P:(bt + 1) * P])
            xts.append(xt_t)

        accs = [acc_pool.tile([P, dout_s], f32, tag="acc",
                              name=f"acc_{i}")
                for i in range(nbt)]

        for c in range(nch):
            w1_t = w_pool.tile([P, it_n, ch], f32r, tag="w1c")
            nc.sync.dma_start(
                w1_t[:], w1_v[:, :, c * ch:(c + 1) * ch])
            w2_t = w_pool.tile([P, it_n, ch], f32r, tag="w2c")
            nc.sync.dma_start(
                w2_t[:], w2_v[:, :, c * ch:(c + 1) * ch])
            if with_b1:
                b1r = ep_pool.tile([1, ch], f32r, tag="b1r")
                nc.sync.dma_start(
                    b1r[:], b1_d[None, c * ch:(c + 1) * ch])

            for bt in range(nbt):
                ph1 = ps1_pool.tile([P, ch], f32, tag="ph1")
                for it in range(it_n):
                    nc.tensor.matmul(
                        ph1[:],
                        lhsT=xts[bt][:, it, :],
                        rhs=w1_t[:, it, :],
                        start=(it == 0),
                        stop=(it == it_n - 1 and not with_b1),
                    )
                if with_b1:
                    nc.tensor.matmul(ph1[:], lhsT=ones_t[:], rhs=b1r[:],
                                     start=False, stop=True)
                ph2 = ps2_pool.tile([P, ch], f32, tag="ph2")
                for it in range(it_n):
                    nc.tensor.matmul(
                        ph2[:],
                        lhsT=xts[bt][:, it, :],
                        rhs=w2_t[:, it, :],
                        start=(it == 0),
                        stop=(it == it_n - 1),
                    )

                h2s = ep_pool.tile([P, ch], f32, tag="h2s")
                nc.scalar.copy(h2s[:], ph2[:])
                p_t = ep_pool.tile([P, ch], f32, tag="pt")
                # p = relu(h1) * h2  in one DVE op
                nc.vector.scalar_tensor_tensor(
                    out=p_t[:], in0=ph1[:], scalar=0.0, in1=h2s[:],
                    op0=mybir.AluOpType.max, op1=mybir.AluOpType.mult)
                nc.vector.tensor_reduce(
                    out=accs[bt][:, c * o_ch:(c + 1) * o_ch],
                    in_=p_t[:].rearrange("p (o k) -> p o k", k=k),
                    axis=mybir.AxisListType.X,
                    op=mybir.AluOpType.add)

        for bt in range(nbt):
            if with_b2:
                nc.vector.tensor_add(accs[bt][:], accs[bt][:], b2bc[:])
            nc.sync.dma_start(
                out_d[bt * P:(bt + 1) * P, :],
                accs[bt][:])

    nc.compile()
    nc.m = get_hw_module(nc.m)
    return nc


def shard_inputs(x, w1, b1, w2, b2, n_cores=N_CORES):
    b_dim, din = x.shape
    _, dout, k = w1.shape
    bs = b_dim // B_GROUPS
    ds = dout // D_GROUPS
    xt = round_f32r(np.ascontiguousarray(np.asarray(x, np.float32).T))
    w1s = [round_f32r(w1[:, c * ds:(c + 1) * ds, :]).reshape(din, ds * k)
           for c in range(D_GROUPS)]
    w2s = [round_f32r(w2[:, c * ds:(c + 1) * ds, :]).reshape(din, ds * k)
           for c in range(D_GROUPS)]
    b1s = [round_f32r(b1[c * ds:(c + 1) * ds, :]).reshape(ds * k)
           for c in range(D_GROUPS)]
    b2s = [np.ascontiguousarray(b2[c * ds:(c + 1) * ds], dtype=np.float32)
           for c in range(D_GROUPS)]
    in_maps = []
    for cid in range(n_cores):
        r, c = divmod(cid, D_GROUPS)
        in_maps.append({
            "xt": np.ascontiguousarray(xt[:, r * bs:(r + 1) * bs]),
            "w1s": w1s[c],
            "w2s": w2s[c],
            "b1s": b1s[c],
            "b2s": b2s[c],
        })
    return in_maps


def unshard_output(results, b_dim, dout):
    bs = b_dim // B_GROUPS
    ds = dout // D_GROUPS
    out = np.empty((b_dim, dout), dtype=np.float32)
    for cid in range(N_CORES):
        r, c = divmod(cid, D_GROUPS)
        out[r * bs:(r + 1) * bs, c * ds:(c + 1) * ds] = results[cid]["out"]
    return out


_PROGRAM_CACHE = {}


def _get_program(din, b, dout_s, k, with_b1, with_b2):
    key = (din, b, dout_s, k, with_b1, with_b2)
    if key not in _PROGRAM_CACHE:
        _PROGRAM_CACHE[key] = build_program(
            din, b, dout_s, k, with_b1, with_b2)
    return _PROGRAM_CACHE[key]


class ParallelRunner:
    """Dispatch the per-core NEFF to each NeuronCore via its own jit so the
    8 executions overlap. (run_bass_kernel_spmd's shard_map path serializes
    the per-device executes through the axon proxy — measured 8x slower
    wall-clock for identical device work.)"""

    def __init__(self, nc, n_cores=N_CORES):
        import jax
        from concourse import bass2jax
        bass2jax.install_neuronx_cc_hook()
        self.jax = jax
        self.n_cores = n_cores
        part = nc.partition_id_tensor.name if nc.partition_id_tensor else None

        in_names, out_names, out_avals, zero_outs = [], [], [], []
        for alloc in nc.m.functions[0].allocations:
            if not isinstance(alloc, mybir.MemoryLocationSet):
                continue
            name = alloc.memorylocations[0].name
            if alloc.kind == "ExternalInput":
                if name != part:
                    in_names.append(name)
            elif alloc.kind == "ExternalOutput":
                out_names.append(name)
                shape = tuple(alloc.tensor_shape)
                dtype = mybir.dt.np(alloc.dtype)
                out_avals.append(jax.core.ShapedArray(shape, dtype))
                zero_outs.append(np.zeros(shape, dtype))
        self.in_names, self.out_names = in_names, out_names
        all_names = in_names + out_names + ([part] if part else [])

        def _body(*args):
            operands = list(args)
            if part is not None:
                operands.append(bass2jax.partition_id_tensor())
            return tuple(bass2jax._bass_exec_p.bind(
                *operands,
                out_avals=tuple(out_avals),
                in_names=tuple(all_names),
                out_names=tuple(out_names),
                lowering_input_output_aliases=(),
                sim_require_finite=True,
                sim_require_nnan=True,
                nc=nc,
            ))

        self.devices = jax.devices()[:n_cores]
        self.fns = [jax.jit(_body, device=d, keep_unused=True)
                    for d in self.devices]
        self.zero_dev = [
            [jax.device_put(z, d) for z in zero_outs] for d in self.devices]

    def __call__(self, in_maps):
        outs = []
        for c in range(self.n_cores):
            args = [self.jax.device_put(np.asarray(in_maps[c][n]),
                                        self.devices[c])
                    for n in self.in_names]
            outs.append(self.fns[c](*args, *self.zero_dev[c]))
        self.jax.block_until_ready(outs)
        return [{n: np.asarray(outs[c][i])
                 for i, n in enumerate(self.out_names)}
                for c in range(self.n_cores)]


_RUNNER_CACHE = {}


def _run(nc, in_maps):
    key = id(nc)
    try:
        if key not in _RUNNER_CACHE:
            _RUNNER_CACHE[key] = ParallelRunner(nc)
        return _RUNNER_CACHE[key](in_maps)
    except Exception:
        res = run_bass_kernel_spmd(nc, in_maps,
                                   core_ids=list(range(N_CORES)))
        return res.results


def kernel(x, w1, b1, w2, b2):
    x = np.asarray(x, dtype=np.float32)
    w1 = np.asarray(w1, dtype=np.float32)
    b1 = np.asarray(b1, dtype=np.float32)
    w2 = np.asarray(w2, dtype=np.float32)
    b2 = np.asarray(b2, dtype=np.float32)

    b_dim, din = x.shape
    _, dout, k = w1.shape
    bs = b_dim // B_GROUPS
    ds = dout // D_GROUPS

    nc = _get_program(din, bs, ds, k,
                      bool(np.any(b1)), bool(np.any(b2)))
    in_maps = shard_inputs(x, w1, b1, w2, b2)
    results = _run(nc, in_maps)
    return np.ascontiguousarray(unshard_output(results, b_dim, dout))


# revision 4
# speedup vs baseline: 94.6158x; 1.1890x over previous
"""TRN2 Bass kernel for nn_Mix2Layer (dense MLP mixture).

Reference computation (all fp32):
    g   = relu(einsum('bi,iok->bok', x, w1) + b1)        # [B, DOUT, K]
    out = einsum('bi,iok,bok->bo', x, w2, g) + b2        # [B, DOUT]

Strategy: 2x4 grid over the 8 NeuronCores — batch B split in 2 groups of
1024 rows, DOUT split in 4 shards of 512 (the bok intermediate never
leaves its core). On each core both einsums are plain matmuls of the
core's x rows [1024, DIN] against the shard's weights flattened to
[DIN, DS*K], run on the PE array in float32r — the PE fast path for
4-byte floats (1 cycle/row when the moving dim is >=256, i.e.
bf16-speed). float32r keeps 11 explicit mantissa bits (measured on
hardware: round-to-nearest-even at 11 bits on both operands reproduces
the PE result to 1e-7), giving ~2e-4 relative error overall.

The matmul operands are pre-rounded to the fp32r grid on the HOST, so all
tensors are declared float32r end-to-end and every DMA is a fast same-dtype
HWDGE transfer (the gpsimd cast-DMA path measured ~100x below line rate).

vs the earlier 1x8 dout-only sharding (B processed in two halves per
core): the xT shard (64 KB/partition) now stays resident for the whole
kernel, which removes the 31 us PE stall at the half boundary (xT
reload) seen in the NTFF trace, and the xT load is split per b-tile so
the first matmuls start ~10 us in instead of ~44 us.

Per-core loop structure:
  xt_bt[8] <- xT b-tiles (8 x 1 MB DMAs, overlap chunk-0 weight DMAs)
  for ok_chunk (256 cols of DS*K=8192):   # w1/w2 chunk tiles double-buffered
    for b_tile (8 x 128 rows):
      psum_h1 = sum_i xt_bt.T @ w1_chunk_i   (16 fp32r matmuls)
      psum_h2 = sum_i xt_bt.T @ w2_chunk_i
      h2s     = copy(psum_h2)                 (ScalarE)
      p       = relu(psum_h1) * h2s           (VectorE, one fused op)
      acc[b_tile][:, chunk] = reduce_k(p)     (VectorE, 3D-AP reduce)
  DMA acc tiles -> out rows
"""
import numpy as np

import concourse.bass as bass
import concourse.tile as tile
import concourse.mybir as mybir
from concourse import bacc
from concourse.bass_interp import get_hw_module
from concourse.bass_utils import run_bass_kernel_spmd

P = 128
f32 = mybir.dt.float32
f32r = mybir.dt.float32r

N_CORES = 8
B_GROUPS = 2   # batch split across cores
D_GROUPS = 4   # dout split across cores
F32R_MANT_BITS = 11


def round_f32r(a):
    """Round fp32 array to the fp32r grid (11 explicit mantissa bits, RNE)."""
    a = np.ascontiguousarray(a, dtype=np.float32)
    bits = a.view(np.uint32)
    shift = np.uint32(23 - F32R_MANT_BITS)
    lsb = np.uint32(1) << shift
    half = lsb >> np.uint32(1)
    rounded = (bits + half + ((bits >> shift) & np.uint32(1)) - np.uint32(1)) \
        & ~np.uint32(lsb - np.uint32(1))
    return rounded.view(np.float32)


def build_program(din, b, dout_s, k, with_b1, with_b2, ch=256,
                  num_devices=N_CORES):
    """Build + schedule + compile the per-core Bass program.

    din: contraction dim; b: per-core batch rows; dout_s: per-core dout
    shard; k: mixture. ch: ok-chunk width (matmul free dim; >=256 keeps
    fp32r at 1 cyc/row).
    """
    okw = dout_s * k
    assert din % P == 0 and b % P == 0 and okw % ch == 0
    assert ch % k == 0
    it_n = din // P
    nbt = b // P
    nch = okw // ch
    o_ch = ch // k

    nc = bacc.Bacc("TRN2", target_bir_lowering=False, debug=False,
                   enable_asserts=True, num_devices=num_devices)
    xt_d = nc.dram_tensor("xt", [din, b], f32r, kind="ExternalInput").ap()
    w1_d = nc.dram_tensor("w1s", [din, okw], f32r, kind="ExternalInput").ap()
    w2_d = nc.dram_tensor("w2s", [din, okw], f32r, kind="ExternalInput").ap()
    b1_d = nc.dram_tensor("b1s", [okw], f32r, kind="ExternalInput").ap()
    b2_d = nc.dram_tensor("b2s", [dout_s], f32, kind="ExternalInput").ap()
    out_d = nc.dram_tensor("out", [b, dout_s], f32, kind="ExternalOutput").ap()

    # DRAM views with i-tiles split out: [p, it, cols]
    xt_v = xt_d.rearrange("(t p) b -> p t b", p=P)
    w1_v = w1_d.rearrange("(t p) n -> p t n", p=P)
    w2_v = w2_d.rearrange("(t p) n -> p t n", p=P)

    from contextlib import ExitStack
    with tile.TileContext(nc) as tc, ExitStack() as ctx:
        xt_pool = ctx.enter_context(tc.tile_pool(name="xt", bufs=nbt))
        w_pool = ctx.enter_context(tc.tile_pool(name="w", bufs=2))
        acc_pool = ctx.enter_context(tc.tile_pool(name="acc", bufs=nbt))
        ep_pool = ctx.enter_context(tc.tile_pool(name="ep", bufs=3))
        const_pool = ctx.enter_context(tc.tile_pool(name="const", bufs=1))
        ps1_pool = ctx.enter_context(
            tc.tile_pool(name="ps1", bufs=3, space="PSUM"))
        ps2_pool = ctx.enter_context(
            tc.tile_pool(name="ps2", bufs=3, space="PSUM"))

        if with_b1:
            ones_t = const_pool.tile([1, P], f32r, tag="ones")
            nc.any.memset(ones_t[:], 1.0)
        if with_b2:
            b2bc = const_pool.tile([P, dout_s], f32, tag="b2bc")
            nc.gpsimd.dma_start(b2bc[:],
                                b2_d[None, :].broadcast_to([P, dout_s]))

        # xT resident for the whole kernel, loaded per b-tile so chunk-0
        # matmuls start as soon as their own slice lands. Issue order and
        # ring placement matter: the sync-HWDGE ring drains in issue
        # order, so put only what the first matmul group needs (xt_bt0 +
        # chunk-0 weights) there, and stream the remaining xt tiles on
        # the scalar-engine HWDGE ring in parallel.
        xts = []
        xt_t = xt_pool.tile([P, it_n, P], f32r, tag="xtb", name="xt_0")
        nc.sync.dma_start(xt_t[:], xt_v[:, :, 0:P])
        xts.append(xt_t)

        w1_t0 = w_pool.tile([P, it_n, ch], f32r, tag="w1c")
        nc.sync.dma_start(w1_t0[:], w1_v[:, :, 0:ch])
        w2_t0 = w_pool.tile([P, it_n, ch], f32r, tag="w2c")
        nc.sync.dma_start(w2_t0[:], w2_v[:, :, 0:ch])

        for bt in range(1, nbt):
            xt_t = xt_pool.tile([P, it_n, P], f32r, tag="xtb",
                                name=f"xt_{bt}")
            nc.scalar.dma_start(xt_t[:], xt_v[:, :, bt * P:(bt + 1) * P])
            xts.append(xt_t)

        accs = [acc_pool.tile([P, dout_s], f32, tag="acc",
                              name=f"acc_{i}")
                for i in range(nbt)]

        for c in range(nch):
            if c == 0:
                w1_t, w2_t = w1_t0, w2_t0
            else:
                w1_t = w_pool.tile([P, it_n, ch], f32r, tag="w1c")
                nc.sync.dma_start(
                    w1_t[:], w1_v[:, :, c * ch:(c + 1) * ch])
                w2_t = w_pool.tile([P, it_n, ch], f32r, tag="w2c")
                nc.sync.dma_start(
                    w2_t[:], w2_v[:, :, c * ch:(c + 1) * ch])
            if with_b1:
                b1r = ep_pool.tile([1, ch], f32r, tag="b1r")
                nc.sync.dma_start(
                    b1r[:], b1_d[None, c * ch:(c + 1) * ch])

            for bt in range(nbt):
                ph1 = ps1_pool.tile([P, ch], f32, tag="ph1")
                for it in range(it_n):
                    nc.tensor.matmul(
                        ph1[:],
                        lhsT=xts[bt][:, it, :],
                        rhs=w1_t[:, it, :],
                        start=(it == 0),
                        stop=(it == it_n - 1 and not with_b1),
                    )
                if with_b1:
                    nc.tensor.matmul(ph1[:], lhsT=ones_t[:], rhs=b1r[:],
                                     start=False, stop=True)
                ph2 = ps2_pool.tile([P, ch], f32, tag="ph2")
                for it in range(it_n):
                    nc.tensor.matmul(
                        ph2[:],
                        lhsT=xts[bt][:, it, :],
                        rhs=w2_t[:, it, :],
                        start=(it == 0),
                        stop=(it == it_n - 1),
                    )

                h2s = ep_pool.tile([P, ch], f32, tag="h2s")
                nc.scalar.copy(h2s[:], ph2[:])
                p_t = ep_pool.tile([P, ch], f32, tag="pt")
                # p = relu(h1) * h2  in one DVE op
                nc.vector.scalar_tensor_tensor(
                    out=p_t[:], in0=ph1[:], scalar=0.0, in1=h2s[:],
                    op0=mybir.AluOpType.max, op1=mybir.AluOpType.mult)
                nc.vector.tensor_reduce(
                    out=accs[bt][:, c * o_ch:(c + 1) * o_ch],
                    in_=p_t[:].rearrange("p (o k) -> p o k", k=k),
                    axis=mybir.AxisListType.X,
                    op=mybir.AluOpType.add)

        for bt in range(nbt):
            if with_b2:
                nc.vector.tensor_add(accs[bt][:], accs[bt][:], b2bc[:])
            nc.scalar.dma_start(
                out_d[bt * P:(bt + 1) * P, :],
                accs[bt][:])

    nc.compile()
    nc.m = get_hw_module(nc.m)
    return nc


def shard_inputs(x, w1, b1, w2, b2, n_cores=N_CORES):
    b_dim, din = x.shape
    _, dout, k = w1.shape
    bs = b_dim // B_GROUPS
    ds = dout // D_GROUPS
    xt = round_f32r(np.ascontiguousarray(np.asarray(x, np.float32).T))
    w1s = [round_f32r(w1[:, c * ds:(c + 1) * ds, :]).reshape(din, ds * k)
           for c in range(D_GROUPS)]
    w2s = [round_f32r(w2[:, c * ds:(c + 1) * ds, :]).reshape(din, ds * k)
           for c in range(D_GROUPS)]
    b1s = [round_f32r(b1[c * ds:(c + 1) * ds, :]).reshape(ds * k)
           for c in range(D_GROUPS)]
    b2s = [np.ascontiguousarray(b2[c * ds:(c + 1) * ds], dtype=np.float32)
           for c in range(D_GROUPS)]
    in_maps = []
    for cid in range(n_cores):
        r, c = divmod(cid, D_GROUPS)
        in_maps.append({
            "xt": np.ascontiguousarray(xt[:, r * bs:(r + 1) * bs]),
            "w1s": w1s[c],
            "w2s": w2s[c],
            "b1s": b1s[c],
            "b2s": b2s[c],
        })
    return in_maps


def unshard_output(results, b_dim, dout):
    bs = b_dim // B_GROUPS
    ds = dout // D_GROUPS
    out = np.empty((b_dim, dout), dtype=np.float32)
    for cid in range(N_CORES):
        r, c = divmod(cid, D_GROUPS)
        out[r * bs:(r + 1) * bs, c * ds:(c + 1) * ds] = results[cid]["out"]
    return out


_PROGRAM_CACHE = {}


def _get_program(din, b, dout_s, k, with_b1, with_b2):
    key = (din, b, dout_s, k, with_b1, with_b2)
    if key not in _PROGRAM_CACHE:
        _PROGRAM_CACHE[key] = build_program(
            din, b, dout_s, k, with_b1, with_b2)
    return _PROGRAM_CACHE[key]


class ParallelRunner:
    """Dispatch the per-core NEFF to each NeuronCore via its own jit so the
    8 executions overlap. (run_bass_kernel_spmd's shard_map path serializes
    the per-device executes through the axon proxy — measured 8x slower
    wall-clock for identical device work.)"""

    def __init__(self, nc, n_cores=N_CORES):
        import jax
        from concourse import bass2jax
        bass2jax.install_neuronx_cc_hook()
        self.jax = jax
        self.n_cores = n_cores
        part = nc.partition_id_tensor.name if nc.partition_id_tensor else None

        in_names, out_names, out_avals, zero_outs = [], [], [], []
        for alloc in nc.m.functions[0].allocations:
            if not isinstance(alloc, mybir.MemoryLocationSet):
                continue
            name = alloc.memorylocations[0].name
            if alloc.kind == "ExternalInput":
                if name != part:
                    in_names.append(name)
            elif alloc.kind == "ExternalOutput":
                out_names.append(name)
                shape = tuple(alloc.tensor_shape)
                dtype = mybir.dt.np(alloc.dtype)
                out_avals.append(jax.core.ShapedArray(shape, dtype))
                zero_outs.append(np.zeros(shape, dtype))
        self.in_names, self.out_names = in_names, out_names
        all_names = in_names + out_names + ([part] if part else [])

        def _body(*args):
            operands = list(args)
            if part is not None:
                operands.append(bass2jax.partition_id_tensor())
            return tuple(bass2jax._bass_exec_p.bind(
                *operands,
                out_avals=tuple(out_avals),
                in_names=tuple(all_names),
                out_names=tuple(out_names),
                lowering_input_output_aliases=(),
                sim_require_finite=True,
                sim_require_nnan=True,
                nc=nc,
            ))

        self.devices = jax.devices()[:n_cores]
        self.fns = [jax.jit(_body, device=d, keep_unused=True)
                    for d in self.devices]
        self.zero_dev = [
            [jax.device_put(z, d) for z in zero_outs] for d in self.devices]

    def __call__(self, in_maps):
        outs = []
        for c in range(self.n_cores):
            args = [self.jax.device_put(np.asarray(in_maps[c][n]),
                                        self.devices[c])
                    for n in self.in_names]
            outs.append(self.fns[c](*args, *self.zero_dev[c]))
        self.jax.block_until_ready(outs)
        return [{n: np.asarray(outs[c][i])
                 for i, n in enumerate(self.out_names)}
                for c in range(self.n_cores)]


_RUNNER_CACHE = {}


def _run(nc, in_maps):
    key = id(nc)
    try:
        if key not in _RUNNER_CACHE:
            _RUNNER_CACHE[key] = ParallelRunner(nc)
        return _RUNNER_CACHE[key](in_maps)
    except Exception:
        res = run_bass_kernel_spmd(nc, in_maps,
                                   core_ids=list(range(N_CORES)))
        return res.results


def kernel(x, w1, b1, w2, b2):
    x = np.asarray(x, dtype=np.float32)
    w1 = np.asarray(w1, dtype=np.float32)
    b1 = np.asarray(b1, dtype=np.float32)
    w2 = np.asarray(w2, dtype=np.float32)
    b2 = np.asarray(b2, dtype=np.float32)

    b_dim, din = x.shape
    _, dout, k = w1.shape
    bs = b_dim // B_GROUPS
    ds = dout // D_GROUPS

    nc = _get_program(din, bs, ds, k,
                      bool(np.any(b1)), bool(np.any(b2)))
    in_maps = shard_inputs(x, w1, b1, w2, b2)
    results = _run(nc, in_maps)
    return np.ascontiguousarray(unshard_output(results, b_dim, dout))
